# revision 15
# baseline (speedup 1.0000x reference)
"""Trainium2 Bass kernel for the Synthesizer-style mixed attention module.

Math (per reference):
  queries = query @ Wq + bq                  [B,H,S,HD]
  values  = value @ Wv + bv                  [B,S,H,HD]
  rand_attn = softmax(random_mat, -1)        [H,S,S]
  dense = relu(queries @ W1 + b1) @ W2 + b2  [B,H,S,S]
  mixed = softmax(s1*rand_attn + s2*dense)   s1 = a1/(a1+a2), s2 = a2/(a1+a2)
  out   = (mixed @ values) @ Wo + bo         [B,S,D]

Sharding: tensor-parallel over heads, 2 heads per core on 8 cores. Each core
computes a full [B,S,D] partial of the output projection for its 2 heads;
the host sums the 8 partials and adds bo.

Device-side layout is "transposed attention": all attention tensors live as
[t(keys) on partitions, q(queries) on free].  Softmax denominators are
obtained without cross-partition reductions:
  - rand branch: Zr = ones^T @ exp(rm^T) via a [128,1]-ones matmul,
    1/Zr via ACT ln->exp(-x), broadcast over partitions with gpsimd.
  - mixed softmax: a constant all-ones row is appended to the values matrix
    (stationary operand of attn@V), so row 64 of the attention output PSUM is
    exactly Z_q; normalization happens on the small [64, S] output.
  - no max-subtraction: logits are bounded by |s1| + |s2|*O(0.1) (softmaxed
    rand term is in (0,1); dense logits are O(0.1) by construction), safe for
    fp32 exp up to |s1| ~ 80.
  E = exp(logits) is formed as exp(s2*dense + s2*b2) * exp(s1*rand_attn); the
  second factor is per-head and reused across the batch.
"""

import sys

sys.path.insert(0, "/opt/trn_rl_repo")

from contextlib import ExitStack

import numpy as np

import concourse.bass as bass
import concourse.tile as tile
from concourse import bacc, mybir
from concourse.bass_utils import run_bass_kernel_spmd

B, S, D, H, HD = 4, 1024, 1024, 16, 64
NCORES = 8
HPC = H // NCORES  # heads per core = 2
HD2 = HPC * HD  # 128
P = 128
KC = D // P  # 8 contraction chunks for the projections
TC = S // P  # 8 key(t) chunks
QC = S // 512  # 2 free-dim halves per 1024

F32 = mybir.dt.float32
F16 = mybir.dt.float16
BF16 = mybir.dt.bfloat16
AF = mybir.ActivationFunctionType
ALU = mybir.AluOpType

NP_BF16 = mybir.dt.np(BF16)

_CACHE = {}


def _build_program():
    nc = bacc.Bacc("TRN2", target_bir_lowering=False, debug=False, num_devices=NCORES)

    qT = nc.dram_tensor("qT", [B, D, S], F16, kind="ExternalInput")
    vT = nc.dram_tensor("vT", [B, D, S], F16, kind="ExternalInput")
    rmT = nc.dram_tensor("rmT", [HPC, S, S], BF16, kind="ExternalInput")
    wq = nc.dram_tensor("wq", [D, HD2], F16, kind="ExternalInput")
    wv = nc.dram_tensor("wv", [D, HD2], F16, kind="ExternalInput")
    w1d = nc.dram_tensor("w1d", [P, HD], F16, kind="ExternalInput")  # [W1;W1]
    w2 = nc.dram_tensor("w2", [HD, S], F16, kind="ExternalInput")
    wo = nc.dram_tensor("wo", [HD2, D], F32, kind="ExternalInput")
    bq = nc.dram_tensor("bq", [HD2, 1], F32, kind="ExternalInput")
    bv = nc.dram_tensor("bv", [HD2, 1], F32, kind="ExternalInput")
    b1 = nc.dram_tensor("b1", [HD, 1], F32, kind="ExternalInput")
    b2c = nc.dram_tensor("b2c", [P, TC], F32, kind="ExternalInput")
    alpha = nc.dram_tensor("alpha", [1, 2], F32, kind="ExternalInput")
    identd = nc.dram_tensor("identd", [P, P], F32, kind="ExternalInput")
    out = nc.dram_tensor("out", [B, S, D], F16, kind="ExternalOutput")

    with tile.TileContext(nc) as tc, ExitStack() as ctx:
        consts = ctx.enter_context(tc.tile_pool(name="consts", bufs=1))
        persist = ctx.enter_context(tc.tile_pool(name="persist", bufs=1))
        qvstage = ctx.enter_context(tc.tile_pool(name="qvstage", bufs=2))
        rmstage = ctx.enter_context(tc.tile_pool(name="rmstage", bufs=3))
        small = ctx.enter_context(tc.tile_pool(name="small", bufs=2))
        ework = ctx.enter_context(tc.tile_pool(name="ework", bufs=3))
        norm = ctx.enter_context(tc.tile_pool(name="norm", bufs=2))
        nscr = ctx.enter_context(tc.tile_pool(name="nscr", bufs=1))
        dscr = ctx.enter_context(tc.tile_pool(name="dscr", bufs=2, space="DRAM"))
        ps_mm = ctx.enter_context(tc.tile_pool(name="ps_mm", bufs=2, space="PSUM"))
        ps_acc = ctx.enter_context(tc.tile_pool(name="ps_acc", bufs=2, space="PSUM"))

        # ---- constants / weights -------------------------------------------
        # alpha scalars, replicated on all 128 partitions from the start
        al = consts.tile([P, 2], F32, tag="al")
        nc.sync.dma_start(al[:], alpha[:].to_broadcast((P, 2)))
        denom = consts.tile([P, 1], F32, tag="denom")
        nc.vector.tensor_add(denom[:], al[:, 0:1], al[:, 1:2])
        rden = consts.tile([P, 1], F32, tag="rden")
        nc.vector.reciprocal(rden[:], denom[:])
        s1 = consts.tile([P, 1], F32, tag="s1")
        nc.vector.tensor_mul(s1[:], al[:, 0:1], rden[:])
        s2bc = consts.tile([P, 1], F32, tag="s2bc")
        nc.vector.tensor_mul(s2bc[:], al[:, 1:2], rden[:])

        wq_t = consts.tile([P, KC, HD2], F16, tag="wq")
        nc.sync.dma_start(wq_t[:], wq[:].rearrange("(c p) m -> p c m", p=P))
        wv_t = consts.tile([P, KC, HD2], F16, tag="wv")
        nc.sync.dma_start(wv_t[:], wv[:].rearrange("(c p) m -> p c m", p=P))
        w1d_t = consts.tile([P, HD], F16, tag="w1d")
        nc.sync.dma_start(w1d_t[:], w1d[:])
        w2_ld = consts.tile([HD, S], F16, tag="w2ld")
        nc.sync.dma_start(w2_ld[:], w2[:])
        w2s_flat = consts.tile([HD, S], F16, tag="w2s")
        nc.vector.tensor_tensor(
            w2s_flat[:], w2_ld[:], s2bc[:HD, :].to_broadcast((HD, S)), ALU.mult
        )
        w2s = w2s_flat[:].rearrange("j (c m) -> j c m", c=TC)
        wo_t = consts.tile([HD2, D], F32, tag="wo")
        nc.sync.dma_start(wo_t[:], wo[:])
        # small per-partition constants: bounce through a DVE copy so later
        # tensor_scalar consumers have same-engine deps (TS ISA has few wait
        # slots)
        bcat_ld = consts.tile([P, 4], F32, tag="bcatld")
        nc.sync.dma_start(bcat_ld[:, 0:1], bq[:])
        nc.sync.dma_start(bcat_ld[:, 1:2], bv[:])
        nc.sync.dma_start(bcat_ld[:HD, 2:3], b1[:])
        b2_ld = consts.tile([P, TC], F32, tag="b2ld")
        nc.sync.dma_start(b2_ld[:], b2c[:])
        bcat = consts.tile([P, 4], F32, tag="bcat")
        nc.vector.tensor_copy(bcat[:], bcat_ld[:])
        bq_t = bcat[:, 0:1]
        bv_t = bcat[:, 1:2]
        b1_t = bcat[:HD, 2:3]
        b2s = consts.tile([P, TC], F32, tag="b2s")
        nc.vector.tensor_tensor(
            b2s[:], b2_ld[:], s2bc[:].to_broadcast((P, TC)), ALU.mult
        )

        ones_t = consts.tile([P, 1], BF16, tag="ones")
        nc.vector.memset(ones_t[:], 1.0)
        ident = consts.tile([P, P], F32, tag="ident")
        nc.sync.dma_start(ident[:], identd[:])

        # ---- rand branch: expS_h = exp(s1 * softmax(rm_h)) per head --------
        expS = []
        for h in range(HPC):
            expR = persist.tile([P, TC, S], BF16, tag="expR")  # reused across heads
            zr = ps_acc.tile([1, S], F32, tag="acc")
            for t in range(TC):
                rmt = rmstage.tile([P, S], BF16, tag="rmt")
                nc.sync.dma_start(rmt[:], rmT[h, t * P : (t + 1) * P, :])
                nc.scalar.activation(expR[:, t, :], rmt[:], AF.Exp)
                for q in range(QC):
                    nc.tensor.matmul(
                        zr[:, q * 512 : (q + 1) * 512],
                        lhsT=ones_t[:],
                        rhs=expR[:, t, q * 512 : (q + 1) * 512],
                        start=(t == 0),
                        stop=(t == TC - 1),
                    )
            # u = s1 / Zr  (per-q row), via ln -> exp(-x), then * s1
            lnz = small.tile([1, S], F32, tag="lnz")
            nc.scalar.activation(lnz[:], zr[:], AF.Ln)
            rz = small.tile([1, S], F32, tag="rzr")
            nc.scalar.activation(rz[:], lnz[:], AF.Exp, scale=-1.0)
            u_row = small.tile([1, S], BF16, tag="urow")
            nc.vector.tensor_scalar_mul(u_row[:], rz[:], s1[0:1, :])
            ubc = small.tile([P, S], BF16, tag="ubc")
            u_d = dscr.tile([1, S], BF16, tag="u_d")
            nc.sync.dma_start(u_d[:], u_row[:])
            nc.sync.dma_start(ubc[:], u_d[:].to_broadcast((P, S)))
            expS_h = persist.tile([P, TC, S], BF16, tag=f"expS{h}")
            for t in range(TC):
                sr = ework.tile([P, S], BF16, tag="srtmp")
                nc.vector.tensor_mul(sr[:], expR[:, t, :], ubc[:])
                nc.scalar.activation(expS_h[:, t, :], sr[:], AF.Exp)
            expS.append(expS_h)

        # ---- per-batch ------------------------------------------------------
        qT_sb = persist.tile([HD2, B, S], F16, tag="qTsb")
        for b in range(B):
            # Q/V projections: out [hd2, s] = Wq_c^T @ qT[b]
            q_ps = ps_mm.tile([HD2, S], F32, tag="mm")
            v_ps = ps_mm.tile([HD2, S], F32, tag="mm")
            for half in range(4):
                qstg = qvstage.tile([P, KC // 4, S], F16, tag="qstg")
                nc.sync.dma_start(
                    qstg[:],
                    qT[b, half * 256 : (half + 1) * 256, :].rearrange(
                        "(c p) s -> p c s", p=P
                    ),
                )
                vstg = qvstage.tile([P, KC // 4, S], F16, tag="vstg")
                nc.sync.dma_start(
                    vstg[:],
                    vT[b, half * 256 : (half + 1) * 256, :].rearrange(
                        "(c p) s -> p c s", p=P
                    ),
                )
                for k in range(KC // 4):
                    kc = half * (KC // 4) + k
                    for q in range(QC):
                        sl = slice(q * 512, (q + 1) * 512)
                        nc.tensor.matmul(
                            q_ps[:, sl],
                            lhsT=wq_t[:, kc, :],
                            rhs=qstg[:, k, sl],
                            start=(kc == 0),
                            stop=(kc == KC - 1),
                        )
                        nc.tensor.matmul(
                            v_ps[:, sl],
                            lhsT=wv_t[:, kc, :],
                            rhs=vstg[:, k, sl],
                            start=(kc == 0),
                            stop=(kc == KC - 1),
                        )
            nc.vector.tensor_scalar_add(qT_sb[:, b, :], q_ps[:], bq_t)
            vT_sb = norm.tile([HD2, S], F32, tag="vTsb")
            nc.vector.tensor_scalar_add(vT_sb[:], v_ps[:], bv_t)

            # transpose values into [t, hd] with an appended ones column
            vaug = [
                persist.tile(
                    [P, TC, HD + 1], BF16, tag=f"vaug{b}_{h}", name=f"vaug{b}_{h}"
                )
                for h in range(HPC)
            ]
            for h in range(HPC):
                nc.vector.memset(vaug[h][:, :, HD : HD + 1], 1.0)
            for t in range(TC):
                t_ps = ps_mm.tile([P, P], F32, tag="mm")
                nc.tensor.transpose(t_ps[:], vT_sb[:, t * P : (t + 1) * P], ident[:])
                for h in range(HPC):
                    nc.vector.tensor_copy(
                        vaug[h][:, t, 0:HD], t_ps[:, h * HD : (h + 1) * HD]
                    )

            # ---- per (b, h) slab -------------------------------------------
            onorm = norm.tile([HD2, S], F32, tag="onorm")
            for h in range(HPC):
                # dense branch MLP part 1: a1^T = relu(W1^T @ q_h^T + b1)
                a1_ps = ps_mm.tile([HD, S], F32, tag="mm")
                hs = slice(h * HD, (h + 1) * HD)
                for q in range(QC):
                    sl = slice(q * 512, (q + 1) * 512)
                    nc.tensor.matmul(
                        a1_ps[:, sl],
                        lhsT=w1d_t[hs, :],
                        rhs=qT_sb[hs, b, sl],
                        start=True,
                        stop=True,
                    )
                a1_sb = ework.tile([HD, S], F16, tag="a1")
                nc.vector.tensor_scalar(
                    a1_sb[:], a1_ps[:], b1_t, 0.0, ALU.add, ALU.max
                )

                attn_ps = ps_acc.tile([HD + 1, S], F32, tag="acc")
                for t in range(TC):
                    lg_ps = ps_mm.tile([P, S], F32, tag="mm")
                    for q in range(QC):
                        sl = slice(q * 512, (q + 1) * 512)
                        nc.tensor.matmul(
                            lg_ps[:, sl],
                            lhsT=w2s[:, t, :],
                            rhs=a1_sb[:, sl],
                            start=True,
                            stop=True,
                        )
                    expD = ework.tile([P, S], BF16, tag="expD")
                    nc.scalar.activation(
                        expD[:], lg_ps[:], AF.Exp, bias=b2s[:, t : t + 1]
                    )
                    e_t = ework.tile([P, S], BF16, tag="E")
                    nc.vector.tensor_mul(e_t[:], expD[:], expS[h][:, t, :])
                    for q in range(QC):
                        sl = slice(q * 512, (q + 1) * 512)
                        nc.tensor.matmul(
                            attn_ps[:, sl],
                            lhsT=vaug[h][:, t, :],
                            rhs=e_t[:, sl],
                            start=(t == 0),
                            stop=(t == TC - 1),
                        )

                # normalize: rows 0..63 are unnormalized attn out, row 64 is Z
                zrow = small.tile([1, S], F32, tag="zrow")
                nc.scalar.activation(zrow[:], attn_ps[HD : HD + 1, :], AF.Copy)
                zbc = nscr.tile([HD, S], F32, tag="zbc")
                z_d = dscr.tile([1, S], F32, tag="z_d")
                nc.sync.dma_start(z_d[:], zrow[:])
                nc.sync.dma_start(zbc[:], z_d[:].to_broadcast((HD, S)))
                rzb = nscr.tile([HD, S], F32, tag="rzb")
                rscr = nscr.tile([HD, S], F32, tag="rscr")
                nc.vector.reciprocal_approx_accurate(rzb[:], zbc[:], rscr[:])
                if h == 0:
                    nc.vector.tensor_mul(onorm[0:HD, :], attn_ps[0:HD, :], rzb[:])
                else:
                    nrm1 = nscr.tile([HD, S], F32, tag="nrm1")
                    nc.vector.tensor_mul(nrm1[:], attn_ps[0:HD, :], rzb[:])
                    nc.sync.dma_start(onorm[HD:HD2, :], nrm1[:])

            # ---- output projection: out[b] partial = onorm^T @ Wo ----------
            for qc in range(TC):
                op_ps = ps_mm.tile([P, D], F32, tag="mm")
                for oc in range(QC):
                    sl = slice(oc * 512, (oc + 1) * 512)
                    nc.tensor.matmul(
                        op_ps[:, sl],
                        lhsT=onorm[:, qc * P : (qc + 1) * P],
                        rhs=wo_t[:, sl],
                        start=True,
                        stop=True,
                    )
                ob = ework.tile([P, D], F16, tag="ob")
                if qc % 2 == 0:
                    nc.scalar.activation(ob[:], op_ps[:], AF.Copy)
                else:
                    nc.vector.tensor_copy(ob[:], op_ps[:])
                nc.sync.dma_start(out[b, qc * P : (qc + 1) * P, :], ob[:])

    nc.finalize()
    return nc


def _get_program():
    if "nc" not in _CACHE:
        _CACHE["nc"] = _build_program()
    return _CACHE["nc"]


def _make_in_maps(inputs):
    f32 = lambda x: np.asarray(x, np.float32)
    query = f32(inputs["query"])
    value = f32(inputs["value"])
    Wq = f32(inputs["Wq"])
    Wv = f32(inputs["Wv"])
    W1 = f32(inputs["W1"])
    W2 = f32(inputs["W2"])
    Wo = f32(inputs["Wo"])
    bq = f32(inputs["bq"])
    bv = f32(inputs["bv"])
    b1 = f32(inputs["b1"])
    b2 = f32(inputs["b2"])
    rm = f32(inputs["random_mat"])
    a1 = f32(inputs["alpha_one"])
    a2 = f32(inputs["alpha_two"])

    qT = np.ascontiguousarray(query.transpose(0, 2, 1)).astype(np.float16)
    vTn = np.ascontiguousarray(value.transpose(0, 2, 1)).astype(np.float16)
    w1d = np.concatenate([W1, W1], axis=0).astype(np.float16)
    w2n = W2.astype(np.float16)
    b2cn = np.ascontiguousarray(b2.reshape(TC, P).T)
    alpha = np.array([[a1[0], a2[0]]], np.float32)

    in_maps = []
    for c in range(NCORES):
        h0 = c * HPC
        in_maps.append(
            {
                "qT": qT,
                "vT": vTn,
                "rmT": np.ascontiguousarray(
                    rm[h0 : h0 + HPC].transpose(0, 2, 1)
                ).astype(NP_BF16),
                "wq": np.ascontiguousarray(
                    Wq[:, h0 : h0 + HPC, :].reshape(D, HD2)
                ).astype(np.float16),
                "wv": np.ascontiguousarray(
                    Wv[:, h0 : h0 + HPC, :].reshape(D, HD2)
                ).astype(np.float16),
                "w1d": w1d,
                "w2": w2n,
                "wo": np.ascontiguousarray(Wo[h0 : h0 + HPC].reshape(HD2, D)),
                "bq": np.ascontiguousarray(bq[h0 : h0 + HPC].reshape(HD2, 1)),
                "bv": np.ascontiguousarray(bv[h0 : h0 + HPC].reshape(HD2, 1)),
                "b1": np.ascontiguousarray(b1.reshape(HD, 1)),
                "b2c": b2cn,
                "alpha": alpha,
                "identd": np.eye(P, dtype=np.float32),
            }
        )
    return in_maps


def run(inputs, trace=False):
    """Run the SPMD kernel; returns (output, BassKernelResults)."""
    nc = _get_program()
    in_maps = _make_in_maps(inputs)
    res = run_bass_kernel_spmd(nc, in_maps, list(range(NCORES)), trace=trace)
    bo = np.asarray(inputs["bo"], np.float32)
    acc = np.zeros((B, S, D), np.float32)
    for c in range(NCORES):
        acc += res.results[c]["out"].astype(np.float32)
    acc += bo[None, None, :]
    return acc, res


def kernel(**inputs) -> np.ndarray:
    out, _ = run(inputs, trace=False)
    return out


def timed_run(inputs, iters=10):
    """Execute the compiled kernel repeatedly with device-resident inputs and
    return (output, per-iteration wall times in ns). Mirrors
    bass2jax.run_bass_via_pjrt's multi-core path, minus donation, so repeated
    executions don't re-transfer inputs."""
    import time

    import jax
    from jax.sharding import Mesh, PartitionSpec
    from jax.experimental.shard_map import shard_map

    from concourse import bass2jax as b2j
    from concourse import mybir as _mybir

    b2j.install_neuronx_cc_hook()
    nc = _get_program()
    in_maps = _make_in_maps(inputs)

    partition_name = nc.partition_id_tensor.name if nc.partition_id_tensor else None
    in_names, out_names, out_avals = [], [], []
    for alloc in nc.m.functions[0].allocations:
        if not isinstance(alloc, _mybir.MemoryLocationSet):
            continue
        name = alloc.memorylocations[0].name
        if alloc.kind == "ExternalInput":
            if name != partition_name:
                in_names.append(name)
        elif alloc.kind == "ExternalOutput":
            out_names.append(name)
            out_avals.append(
                jax.core.ShapedArray(
                    tuple(alloc.tensor_shape), _mybir.dt.np(alloc.dtype)
                )
            )
    n_params = len(in_names)
    all_names = in_names + out_names + ([partition_name] if partition_name else [])

    def _body(*args):
        operands = list(args)
        if partition_name is not None:
            operands.append(b2j.partition_id_tensor())
        return tuple(
            b2j._bass_exec_p.bind(
                *operands,
                out_avals=tuple(out_avals),
                in_names=tuple(all_names),
                out_names=tuple(out_names),
                lowering_input_output_aliases=(),
                sim_require_finite=True,
                sim_require_nnan=True,
                nc=nc,
            )
        )

    devices = jax.devices()[:NCORES]
    mesh = Mesh(np.asarray(devices), ("core",))
    nio = n_params + len(out_names)
    sharded = jax.jit(
        shard_map(
            _body,
            mesh=mesh,
            in_specs=(PartitionSpec("core"),) * nio,
            out_specs=(PartitionSpec("core"),) * len(out_names),
            check_rep=False,
        ),
        keep_unused=True,
    )
    concat_in = [
        np.concatenate([np.asarray(in_maps[c][n]) for c in range(NCORES)], axis=0)
        for n in in_names
    ]
    concat_zero = [
        np.zeros((NCORES * a.shape[0], *a.shape[1:]), a.dtype) for a in out_avals
    ]
    sh = jax.sharding.NamedSharding(mesh, PartitionSpec("core"))
    dev_in = [jax.device_put(x, sh) for x in concat_in]
    dev_zero = [jax.device_put(x, sh) for x in concat_zero]

    outs = sharded(*dev_in, *dev_zero)
    jax.block_until_ready(outs)
    times = []
    for _ in range(iters):
        t0 = time.perf_counter_ns()
        outs = sharded(*dev_in, *dev_zero)
        jax.block_until_ready(outs)
        times.append(time.perf_counter_ns() - t0)

    res = [
        {
            n: np.asarray(outs[i]).reshape(NCORES, *out_avals[i].shape)[c]
            for i, n in enumerate(out_names)
        }
        for c in range(NCORES)
    ]
    bo = np.asarray(inputs["bo"], np.float32)
    acc = np.zeros((B, S, D), np.float32)
    for c in range(NCORES):
        acc += res[c]["out"].astype(np.float32)
    acc += bo[None, None, :]
    return acc, times
